# revision 29
# baseline (speedup 1.0000x reference)
"""BinaryConnect 3x3 SAME conv (NHWC, 32x112x112x128 -> 32x112x112x256) on 8 trn2 cores.

Strategy: data-parallel (4 images/core) + 1D Winograd F(2,3) along W.
  - Host: binarize kernel (exact), 1D-Winograd-transform weights
    (coeffs in {+-1, +-1/2, +-3/2}: exact in fp16/fp8) and activations
    (4 xi planes per 2 output cols, computed in fp32, cast fp16; the two
    "edge" planes xi0=d0-d2, xi3=d1-d3 also cast to e4m3 fp8).
  - Device: m[xi] = sum_dh w~[dh,xi] . x~[row+dh, xi] accumulated in PSUM
    (3 dh matmuls per xi, 4 xi per output tile). For xi0/xi3 the dh0+dh1
    matmuls are fused into one fp8 DoubleRow matmul (k-tile steps one
    image row) -> 10 matmul slots per tile instead of direct conv's 9,
    but each covers HALF the pixels (56 col-tiles vs 112 cols): 10/18 of
    direct fp16 work. DR duty 2/10 = 20% stays under the ~22% chip
    power-throttle threshold. fp8 on the edge planes only (their error
    enters one output phase, not two): rel err ~1.76e-2 < 2e-2.
  - Vector engine reconstructs outputs from PSUM via 2 fused
    scalar_tensor_tensor ops per phase: y_even = m0+m1+m2,
    y_odd = m1-m2-m3; writes fp16 even/odd planes, host interleaves.
"""

import os

import numpy as np
import ml_dtypes

import concourse.bass as bass
import concourse.mybir as mybir
import concourse.tile as tile
from concourse import bacc
from concourse.bass_utils import run_bass_kernel_spmd

N_CORES = 8
NPC = 4            # images per core
H = 112
CT = 56            # col tiles (2 out cols each)
HP = 114           # 1 top pad + 112 rows + 1 bottom pad
W6 = 4 * CT        # fp16 x~ row width (4 xi planes)
W8 = 2 * CT        # fp8 x~ row width (xi0, xi3)
CI = 128
CO = 256
TROWS = 8          # output rows per matmul tile
S = TROWS * CT     # 448 matmul free dim (<=512 fp32 PSUM bank)
BROWS = 56         # output rows per band
NB = H // BROWS    # 2 bands per image
BIN = BROWS + 2    # x~ rows per band (1 halo row each side)
TSB = BROWS // TROWS  # 7 tiles per band
NBUF = 4           # x~ band ring depth

F8 = ml_dtypes.float8_e4m3
DR = mybir.MatmulPerfMode.DoubleRow
ADD = mybir.AluOpType.add
MULT = mybir.AluOpType.mult

_nc_cache = None
LAST_RESULT = None


def _build():
    nc = bacc.Bacc(
        "TRN2",
        target_bir_lowering=False,
        debug=False,
        num_devices=N_CORES,
    )
    x8_d = nc.dram_tensor(
        "x8", [CI, NPC, HP, W8], mybir.dt.float8e4, kind="ExternalInput"
    )
    x6_d = nc.dram_tensor(
        "x16", [CI, NPC, HP, W6], mybir.dt.float16, kind="ExternalInput"
    )
    w8_d = nc.dram_tensor(
        "w8", [CI, 2, 2, 2, 128], mybir.dt.float8e4, kind="ExternalInput"
    )
    w6_d = nc.dram_tensor(
        "w16", [CI, 2, 12 * 128], mybir.dt.float16, kind="ExternalInput"
    )
    # out: [co, n, tile(16), phase(2), S]
    o_d = nc.dram_tensor(
        "out_cm", [CO, NPC, NB * TSB * 2 * S], mybir.dt.float16,
        kind="ExternalOutput"
    )
    with tile.TileContext(nc) as tc:
        with (
            tc.tile_pool(name="x8pool", bufs=NBUF) as x8pool,
            tc.tile_pool(name="x16pool", bufs=NBUF) as x16pool,
            tc.tile_pool(name="wpool", bufs=1) as wpool,
            tc.tile_pool(name="tpool", bufs=4) as tpool,
            tc.tile_pool(name="psum", bufs=8, space=bass.MemorySpace.PSUM) as psum,
            tc.tile_pool(name="opool", bufs=4) as opool,
        ):
            # Warmup operand with no DMA dependency (memset) so PE warmup can
            # start right after the framework preamble.
            wta = wpool.tile([CI, S], mybir.dt.float16, tag="wta", name="wta")
            nc.gpsimd.memset(wta[:], 0.0)
            w8t = wpool.tile([CI, 2, 2, 2, 128], mybir.dt.float8e4, tag="w8", name="w8")
            nc.sync.dma_start(w8t[:], w8_d[:])
            w6t = wpool.tile([CI, 2, 12 * 128], mybir.dt.float16, tag="w6", name="w6")
            nc.sync.dma_start(w6t[:], w6_d[:])
            # PE warmup: throwaway matmuls to reach HAM K=8/8 before the real
            # stream begins.
            wu = psum.tile([128, S], mybir.dt.float32, name="ps")
            for _ in range(9):
                nc.tensor.matmul(
                    wu[:], wta[:, 0:128], wta[:, 0:S], start=True, stop=True
                )
            # x~ band ring: band index k = (n*NB + b), buffer k % NBUF.
            xs8, xs6 = {}, {}

            def load_band(n, b):
                k = n * NB + b
                r0 = b * BROWS
                xt8 = x8pool.tile([CI, BIN, W8], mybir.dt.float8e4, name="xb8")
                xt6 = x16pool.tile([CI, BIN, W6], mybir.dt.float16, name="xb6")
                if k == 0:
                    # first band gates early compute: split DMAs into
                    # per-tile chunks so they arrive sooner, spread queues.
                    # alternate issue queues: SP serializes dma_start
                    # issue, and ACT is idle this early -- splitting the
                    # 16 chunk issues across both gets transfers started
                    # sooner and shortens the first-band arrival stall.
                    bounds = [0, 5, 10, 18, 26, 34, 42, 50, BIN]
                    for i in range(8):
                        ra, rb = bounds[i], bounds[i + 1]
                        q = nc.scalar if i % 2 else nc.sync
                        q.dma_start(xt8[:, ra:rb, :], x8_d[:, n, r0 + ra: r0 + rb, :])
                        q.dma_start(xt6[:, ra:rb, :], x6_d[:, n, r0 + ra: r0 + rb, :])
                else:
                    nc.sync.dma_start(xt8[:], x8_d[:, n, r0: r0 + BIN, :])
                    nc.sync.dma_start(xt6[:], x6_d[:, n, r0: r0 + BIN, :])
                xs8[n, b] = xt8
                xs6[n, b] = xt6

            # prefetch the first NBUF bands up front
            order = [(n, b) for n in range(NPC) for b in range(NB)]
            for (n, b) in order[:NBUF]:
                load_band(n, b)

            def emit_group(n, b, t, half, otb, fast_tail=False):
                s8 = xs8[n, b]
                s6 = xs6[n, b]
                r0 = t * TROWS  # band-local x~ row base
                ps = [psum.tile([128, S], mybir.dt.float32, name="ps")
                      for _ in range(4)]
                # Mode-transition pairing: half-0 groups run the fp8 DR
                # slots FIRST, half-1 groups run them LAST, so consecutive
                # groups meet fp16->fp16 and DR->DR at their boundary --
                # two fp8<->fp16 PE transitions per tile instead of four.
                # Accumulation flags move with the order: whichever of the
                # DR / fp16-dh2 slot runs first owns start=True.
                dr_last = (half == 1) and not fast_tail

                def emit_drs():
                    for si in range(2):
                        nat = s8[:, r0: r0 + TROWS, si * CT: si * CT + CT]
                        pstep = nat.ap[0][0]
                        rhs = bass.AP(
                            nat.tensor, r0 * W8 + si * CT,
                            [[pstep, CI], [W8, 2], [W8, TROWS], [1, CT]],
                        )
                        xi = 0 if si == 0 else 3
                        nc.tensor.matmul(
                            ps[xi][:], w8t[:, half, si, :, :], rhs,
                            start=not dr_last, stop=dr_last, perf_mode=DR,
                        )

                if not dr_last:
                    emit_drs()
                # fp16 slots, ordered so m0/m3/m2 complete early (the
                # PSUM->SBUF staging chains hang off m2 and m1). For the
                # very last tile, finish the ACT-staged banks (m2, m1)
                # mid-group instead so the post-matmul chain is shortest.
                e03 = dr_last  # xi0/xi3 dh2 opens its bank when DR is last
                if fast_tail:
                    f16 = [(2, 0, True, False), (2, 1, False, False),
                           (2, 2, False, True), (1, 0, True, False),
                           (1, 1, False, False), (1, 2, False, True),
                           (0, 2, e03, not e03), (3, 2, e03, not e03)]
                else:
                    f16 = [(0, 2, e03, not e03), (3, 2, e03, not e03),
                           (2, 0, True, False), (2, 1, False, False),
                           (2, 2, False, True), (1, 0, True, False),
                           (1, 1, False, False), (1, 2, False, True)]
                for (xi, dh, st_, sp_) in f16:
                    idx = xi * 3 + dh
                    rhs = s6[:, r0 + dh: r0 + dh + TROWS,
                             xi * CT: xi * CT + CT]
                    nc.tensor.matmul(
                        ps[xi][:], w6t[:, half, idx * 128: idx * 128 + 128],
                        rhs, start=st_, stop=sp_,
                    )
                if dr_last:
                    emit_drs()
                # inverse transform: even = m0+m1+m2, odd = m1-(m2+m3).
                # Engine constraints: ops read at most ONE PSUM operand and
                # gpsimd reads none. ACT stages m2/m1 to SBUF, vector does
                # the PSUM-fused adds, gpsimd the SBUF-only finals.
                ot = otb[:, t, :, :]
                tm = tpool.tile([128, 4, S], mybir.dt.float16, name="tm")
                c2, c1, s1, s2 = (tm[:, j, :] for j in range(4))
                nc.scalar.copy(c2, ps[2][:])
                nc.scalar.copy(c1, ps[1][:])
                nc.vector.scalar_tensor_tensor(
                    s2, c2, 1.0, ps[3][:], MULT, ADD)
                nc.vector.scalar_tensor_tensor(
                    s1, c1, 1.0, ps[0][:], MULT, ADD)
                nc.gpsimd.tensor_sub(ot[:, 1, :], c1, s2)
                nc.vector.scalar_tensor_tensor(
                    ot[:, 0, :], s1, 1.0, c2, MULT, ADD)


            nextband = NBUF
            for ki, (n, b) in enumerate(order):
                otb = [opool.tile([128, TSB, 2, S], mybir.dt.float16,
                                  name="otb") for _ in range(2)]
                # last band: per-tile output DMAs so the final drain after
                # the last matmul is one small transfer, not a 4-tile batch.
                last = ki == len(order) - 1
                for t in range(TSB):
                    for half in range(2):
                        ft = last and t == TSB - 1
                        emit_group(n, b, t, half, otb[half], fast_tail=ft)
                        if last and not ft:
                            nc.scalar.dma_start(
                                o_d[half * 128: half * 128 + 128, n,
                                    (b * TSB + t) * 2 * S:
                                    (b * TSB + t + 1) * 2 * S],
                                otb[half][:, t: t + 1, :, :],
                            )
                        elif ft:
                            # ship each phase as its engine finishes it
                            for ph in range(2):
                                nc.scalar.dma_start(
                                    o_d[half * 128: half * 128 + 128, n,
                                        ((b * TSB + t) * 2 + ph) * S:
                                        ((b * TSB + t) * 2 + ph + 1) * S],
                                    otb[half][:, t, ph, :],
                                )
                        elif t in (3, TSB - 1):
                            lo, hi = (0, 4) if t == 3 else (4, TSB)
                            nc.scalar.dma_start(
                                o_d[half * 128: half * 128 + 128, n,
                                    (b * TSB + lo) * 2 * S:
                                    (b * TSB + hi) * 2 * S],
                                otb[half][:, lo:hi, :, :],
                            )
                if nextband < len(order):
                    nn, nb_ = order[nextband]
                    load_band(nn, nb_)
                    nextband += 1
    nc.compile()
    return nc


def _get_nc():
    global _nc_cache
    if _nc_cache is None:
        _nc_cache = _build()
    return _nc_cache


def kernel(x, kernel):
    global LAST_RESULT
    x = np.asarray(x)
    k = np.asarray(kernel)

    wb = np.where(k >= 0, np.float32(1), np.float32(-1))  # [3,3,128,256]
    # 1D winograd weight transform along W: per dh, 4 xi planes.
    # gt[dh][xi] : [128ci, 256co]
    gt = [[wb[dh, 0],
           (wb[dh, 0] + wb[dh, 1] + wb[dh, 2]) * 0.5,
           (wb[dh, 0] - wb[dh, 1] + wb[dh, 2]) * 0.5,
           wb[dh, 2]] for dh in range(3)]
    # fp8 DR weights: slot si in {0: xi0, 1: xi3}, ktile in {dh0, dh1}
    w8 = np.zeros((CI, 2, 2, 2, 128), np.float32)
    for half in range(2):
        co = slice(half * 128, half * 128 + 128)
        for si, xi in enumerate((0, 3)):
            w8[:, half, si, 0, :] = gt[0][xi][:, co]
            w8[:, half, si, 1, :] = gt[1][xi][:, co]
    w8 = np.ascontiguousarray(w8.astype(F8))
    # fp16 weights: idx = xi*3 + dh
    w16 = np.zeros((CI, 2, 12 * 128), np.float16)
    for half in range(2):
        co = slice(half * 128, half * 128 + 128)
        for xi in range(4):
            for dh in range(3):
                idx = xi * 3 + dh
                w16[:, half, idx * 128: idx * 128 + 128] = gt[dh][xi][:, co]

    # activation transform (fp32), pad H and W by 1.
    xp = np.pad(x, ((0, 0), (1, 1), (1, 1), (0, 0)))  # [32,114,114,128]
    c_idx = np.arange(CT) * 2
    D = [xp[:, :, c_idx + j, :] for j in range(4)]  # [32,114,56,128]
    XT = [D[0] - D[2], D[1] + D[2], D[2] - D[1], D[1] - D[3]]
    # XT[xi]: [32,114,56,128] -> want [128, n, 114, xi, 56]
    xt6 = np.stack(XT, axis=2)          # [32, 114, 4, 56, 128] fp32
    xt8 = np.stack((XT[0], XT[3]), axis=2)  # [32, 114, 2, 56, 128]
    xt6 = np.ascontiguousarray(
        xt6.transpose(4, 0, 1, 2, 3).astype(np.float16))  # [128,32,114,4,56]
    xt8 = np.ascontiguousarray(
        xt8.transpose(4, 0, 1, 2, 3).astype(F8))          # [128,32,114,2,56]

    in_maps = []
    for c in range(N_CORES):
        sl = slice(c * NPC, (c + 1) * NPC)
        in_maps.append({
            "x8": xt8[:, sl].reshape(CI, NPC, HP, W8),
            "x16": xt6[:, sl].reshape(CI, NPC, HP, W6),
            "w8": w8, "w16": w16,
        })

    nc = _get_nc()
    trace = os.environ.get("BCONV_TRACE", "0") == "1"
    kwargs = {}
    if trace and os.environ.get("BCONV_TRACE_CORES", "") == "all":
        kwargs["trace_cores"] = list(range(N_CORES))
    res = run_bass_kernel_spmd(
        nc, in_maps, core_ids=list(range(N_CORES)), trace=trace, **kwargs
    )
    LAST_RESULT = res

    out = np.empty((32, H, H, CO), np.float32)
    for c in range(N_CORES):
        o = res.results[c]["out_cm"].reshape(CO, NPC, NB * TSB, 2, TROWS, CT)
        # row = 7*T + r ; col = 2*c + phase
        y = o.transpose(1, 2, 4, 5, 3, 0).reshape(NPC, H, H, CO)
        out[c * NPC: (c + 1) * NPC] = y.astype(np.float32)
    return out


# revision 30
# speedup vs baseline: 1.1946x; 1.1946x over previous
"""BinaryConnect 3x3 SAME conv (NHWC, 32x112x112x128 -> 32x112x112x256) on 8 trn2 cores.

Strategy: data-parallel (4 images/core) + 1D Winograd F(2,3) along W.
  - Host: binarize kernel (exact), 1D-Winograd-transform weights
    (coeffs in {+-1, +-1/2, +-3/2}: exact in fp16/fp8) and activations
    (4 xi planes per 2 output cols, computed in fp32, cast fp16; the two
    "edge" planes xi0=d0-d2, xi3=d1-d3 also cast to e4m3 fp8).
  - Device: m[xi] = sum_dh w~[dh,xi] . x~[row+dh, xi] accumulated in PSUM
    (3 dh matmuls per xi, 4 xi per output tile). For xi0/xi3 the dh0+dh1
    matmuls are fused into one fp8 DoubleRow matmul (k-tile steps one
    image row) -> 10 matmul slots per tile instead of direct conv's 9,
    but each covers HALF the pixels (56 col-tiles vs 112 cols): 10/18 of
    direct fp16 work. DR duty 2/10 = 20% stays under the ~22% chip
    power-throttle threshold. fp8 on the edge planes only (their error
    enters one output phase, not two): rel err ~1.76e-2 < 2e-2.
  - Vector engine reconstructs outputs from PSUM via 2 fused
    scalar_tensor_tensor ops per phase: y_even = m0+m1+m2,
    y_odd = m1-m2-m3; writes fp16 even/odd planes, host interleaves.
"""

import os

import numpy as np
import ml_dtypes

import concourse.bass as bass
import concourse.mybir as mybir
import concourse.tile as tile
from concourse import bacc
from concourse.bass_utils import run_bass_kernel_spmd

N_CORES = 8
NPC = 4            # images per core
H = 112
CT = 56            # col tiles (2 out cols each)
HP = 114           # 1 top pad + 112 rows + 1 bottom pad
W6 = 4 * CT        # fp16 x~ row width (4 xi planes)
W8 = 2 * CT        # fp8 x~ row width (xi0, xi3)
CI = 128
CO = 256
TROWS = 8          # output rows per matmul tile
S = TROWS * CT     # 448 matmul free dim (<=512 fp32 PSUM bank)
BROWS = 56         # output rows per band
NB = H // BROWS    # 2 bands per image
BIN = BROWS + 2    # x~ rows per band (1 halo row each side)
TSB = BROWS // TROWS  # 7 tiles per band
NBUF = 4           # x~ band ring depth

F8 = ml_dtypes.float8_e4m3
DR = mybir.MatmulPerfMode.DoubleRow
ADD = mybir.AluOpType.add
MULT = mybir.AluOpType.mult

_nc_cache = None
LAST_RESULT = None


def _build():
    nc = bacc.Bacc(
        "TRN2",
        target_bir_lowering=False,
        debug=False,
        num_devices=N_CORES,
    )
    x8_d = nc.dram_tensor(
        "x8", [CI, NPC, HP, W8], mybir.dt.float8e4, kind="ExternalInput"
    )
    x6_d = nc.dram_tensor(
        "x16", [CI, NPC, HP, W6], mybir.dt.float16, kind="ExternalInput"
    )
    w8_d = nc.dram_tensor(
        "w8", [CI, 2, 2, 2, 128], mybir.dt.float8e4, kind="ExternalInput"
    )
    w6_d = nc.dram_tensor(
        "w16", [CI, 2, 12 * 128], mybir.dt.float16, kind="ExternalInput"
    )
    # out: [co, n, tile(16), phase(2), S]
    o_d = nc.dram_tensor(
        "out_cm", [CO, NPC, NB * TSB * 2 * S], mybir.dt.float16,
        kind="ExternalOutput"
    )
    with tile.TileContext(nc) as tc:
        with (
            tc.tile_pool(name="x8pool", bufs=NBUF) as x8pool,
            tc.tile_pool(name="x16pool", bufs=NBUF) as x16pool,
            tc.tile_pool(name="wpool", bufs=1) as wpool,
            tc.tile_pool(name="tpool", bufs=4) as tpool,
            tc.tile_pool(name="psum", bufs=8, space=bass.MemorySpace.PSUM) as psum,
            tc.tile_pool(name="opool", bufs=4) as opool,
        ):
            # Warmup operand with no DMA dependency (memset) so PE warmup can
            # start right after the framework preamble.
            wta = wpool.tile([CI, S], mybir.dt.float16, tag="wta", name="wta")
            nc.gpsimd.memset(wta[:], 0.0)
            w8t = wpool.tile([CI, 2, 2, 2, 128], mybir.dt.float8e4, tag="w8", name="w8")
            nc.sync.dma_start(w8t[:], w8_d[:])
            w6t = wpool.tile([CI, 2, 12 * 128], mybir.dt.float16, tag="w6", name="w6")
            nc.sync.dma_start(w6t[:], w6_d[:])
            # PE warmup: throwaway matmuls to reach HAM K=8/8 before the real
            # stream begins.
            wu = psum.tile([128, S], mybir.dt.float32, name="ps")
            for _ in range(9):
                nc.tensor.matmul(
                    wu[:], wta[:, 0:128], wta[:, 0:S], start=True, stop=True
                )
            # x~ band ring: band index k = (n*NB + b), buffer k % NBUF.
            xs8, xs6 = {}, {}

            def load_band(n, b):
                k = n * NB + b
                r0 = b * BROWS
                xt8 = x8pool.tile([CI, BIN, W8], mybir.dt.float8e4, name="xb8")
                xt6 = x16pool.tile([CI, BIN, W6], mybir.dt.float16, name="xb6")
                if k == 0:
                    # first band gates early compute: split DMAs into
                    # per-tile chunks so they arrive sooner, spread queues.
                    bounds = [0, 5, 10, 18, 26, 34, 42, 50, BIN]
                    for i in range(8):
                        ra, rb = bounds[i], bounds[i + 1]
                        q = nc.scalar if i == 0 else nc.sync
                        q.dma_start(xt8[:, ra:rb, :], x8_d[:, n, r0 + ra: r0 + rb, :])
                        q.dma_start(xt6[:, ra:rb, :], x6_d[:, n, r0 + ra: r0 + rb, :])
                else:
                    nc.sync.dma_start(xt8[:], x8_d[:, n, r0: r0 + BIN, :])
                    nc.sync.dma_start(xt6[:], x6_d[:, n, r0: r0 + BIN, :])
                xs8[n, b] = xt8
                xs6[n, b] = xt6

            # prefetch the first NBUF bands up front
            order = [(n, b) for n in range(NPC) for b in range(NB)]
            for (n, b) in order[:NBUF]:
                load_band(n, b)

            def emit_group(n, b, t, half, otb, fast_tail=False):
                s8 = xs8[n, b]
                s6 = xs6[n, b]
                r0 = t * TROWS  # band-local x~ row base
                ps = [psum.tile([128, S], mybir.dt.float32, name="ps")
                      for _ in range(4)]
                # Mode-transition pairing: half-0 groups run the fp8 DR
                # slots FIRST, half-1 groups run them LAST, so consecutive
                # groups meet fp16->fp16 and DR->DR at their boundary --
                # two fp8<->fp16 PE transitions per tile instead of four.
                # Accumulation flags move with the order: whichever of the
                # DR / fp16-dh2 slot runs first owns start=True.
                dr_last = (half == 1) and not fast_tail

                def emit_drs():
                    for si in range(2):
                        nat = s8[:, r0: r0 + TROWS, si * CT: si * CT + CT]
                        pstep = nat.ap[0][0]
                        rhs = bass.AP(
                            nat.tensor, r0 * W8 + si * CT,
                            [[pstep, CI], [W8, 2], [W8, TROWS], [1, CT]],
                        )
                        xi = 0 if si == 0 else 3
                        nc.tensor.matmul(
                            ps[xi][:], w8t[:, half, si, :, :], rhs,
                            start=not dr_last, stop=dr_last, perf_mode=DR,
                        )

                if not dr_last:
                    emit_drs()
                # fp16 slots, ordered so m0/m3/m2 complete early (the
                # PSUM->SBUF staging chains hang off m2 and m1). For the
                # very last tile, finish the ACT-staged banks (m2, m1)
                # mid-group instead so the post-matmul chain is shortest.
                e03 = dr_last  # xi0/xi3 dh2 opens its bank when DR is last
                if fast_tail:
                    f16 = [(2, 0, True, False), (2, 1, False, False),
                           (2, 2, False, True), (1, 0, True, False),
                           (1, 1, False, False), (1, 2, False, True),
                           (0, 2, e03, not e03), (3, 2, e03, not e03)]
                else:
                    f16 = [(0, 2, e03, not e03), (3, 2, e03, not e03),
                           (2, 0, True, False), (2, 1, False, False),
                           (2, 2, False, True), (1, 0, True, False),
                           (1, 1, False, False), (1, 2, False, True)]
                for (xi, dh, st_, sp_) in f16:
                    idx = xi * 3 + dh
                    rhs = s6[:, r0 + dh: r0 + dh + TROWS,
                             xi * CT: xi * CT + CT]
                    nc.tensor.matmul(
                        ps[xi][:], w6t[:, half, idx * 128: idx * 128 + 128],
                        rhs, start=st_, stop=sp_,
                    )
                if dr_last:
                    emit_drs()
                # inverse transform: even = m0+m1+m2, odd = m1-(m2+m3).
                # Engine constraints: ops read at most ONE PSUM operand and
                # gpsimd reads none. ACT stages m2/m1 to SBUF, vector does
                # the PSUM-fused adds, gpsimd the SBUF-only finals.
                ot = otb[:, t, :, :]
                tm = tpool.tile([128, 4, S], mybir.dt.float16, name="tm")
                c2, c1, s1, s2 = (tm[:, j, :] for j in range(4))
                nc.scalar.copy(c2, ps[2][:])
                nc.scalar.copy(c1, ps[1][:])
                nc.vector.scalar_tensor_tensor(
                    s2, c2, 1.0, ps[3][:], MULT, ADD)
                nc.vector.scalar_tensor_tensor(
                    s1, c1, 1.0, ps[0][:], MULT, ADD)
                nc.gpsimd.tensor_sub(ot[:, 1, :], c1, s2)
                nc.vector.scalar_tensor_tensor(
                    ot[:, 0, :], s1, 1.0, c2, MULT, ADD)


            nextband = NBUF
            for ki, (n, b) in enumerate(order):
                otb = [opool.tile([128, TSB, 2, S], mybir.dt.float16,
                                  name="otb") for _ in range(2)]
                # last band: per-tile output DMAs so the final drain after
                # the last matmul is one small transfer, not a 4-tile batch.
                last = ki == len(order) - 1
                for t in range(TSB):
                    for half in range(2):
                        ft = last and t == TSB - 1
                        emit_group(n, b, t, half, otb[half], fast_tail=ft)
                        if last and not ft:
                            nc.scalar.dma_start(
                                o_d[half * 128: half * 128 + 128, n,
                                    (b * TSB + t) * 2 * S:
                                    (b * TSB + t + 1) * 2 * S],
                                otb[half][:, t: t + 1, :, :],
                            )
                        elif ft:
                            # ship each phase as its engine finishes it
                            for ph in range(2):
                                nc.scalar.dma_start(
                                    o_d[half * 128: half * 128 + 128, n,
                                        ((b * TSB + t) * 2 + ph) * S:
                                        ((b * TSB + t) * 2 + ph + 1) * S],
                                    otb[half][:, t, ph, :],
                                )
                        elif t in (3, TSB - 1):
                            lo, hi = (0, 4) if t == 3 else (4, TSB)
                            nc.scalar.dma_start(
                                o_d[half * 128: half * 128 + 128, n,
                                    (b * TSB + lo) * 2 * S:
                                    (b * TSB + hi) * 2 * S],
                                otb[half][:, lo:hi, :, :],
                            )
                if nextband < len(order):
                    nn, nb_ = order[nextband]
                    load_band(nn, nb_)
                    nextband += 1
    nc.compile()
    return nc


def _get_nc():
    global _nc_cache
    if _nc_cache is None:
        _nc_cache = _build()
    return _nc_cache


def kernel(x, kernel):
    global LAST_RESULT
    x = np.asarray(x)
    k = np.asarray(kernel)

    wb = np.where(k >= 0, np.float32(1), np.float32(-1))  # [3,3,128,256]
    # 1D winograd weight transform along W: per dh, 4 xi planes.
    # gt[dh][xi] : [128ci, 256co]
    gt = [[wb[dh, 0],
           (wb[dh, 0] + wb[dh, 1] + wb[dh, 2]) * 0.5,
           (wb[dh, 0] - wb[dh, 1] + wb[dh, 2]) * 0.5,
           wb[dh, 2]] for dh in range(3)]
    # fp8 DR weights: slot si in {0: xi0, 1: xi3}, ktile in {dh0, dh1}
    w8 = np.zeros((CI, 2, 2, 2, 128), np.float32)
    for half in range(2):
        co = slice(half * 128, half * 128 + 128)
        for si, xi in enumerate((0, 3)):
            w8[:, half, si, 0, :] = gt[0][xi][:, co]
            w8[:, half, si, 1, :] = gt[1][xi][:, co]
    w8 = np.ascontiguousarray(w8.astype(F8))
    # fp16 weights: idx = xi*3 + dh
    w16 = np.zeros((CI, 2, 12 * 128), np.float16)
    for half in range(2):
        co = slice(half * 128, half * 128 + 128)
        for xi in range(4):
            for dh in range(3):
                idx = xi * 3 + dh
                w16[:, half, idx * 128: idx * 128 + 128] = gt[dh][xi][:, co]

    # activation transform (fp32), pad H and W by 1.
    xp = np.pad(x, ((0, 0), (1, 1), (1, 1), (0, 0)))  # [32,114,114,128]
    c_idx = np.arange(CT) * 2
    D = [xp[:, :, c_idx + j, :] for j in range(4)]  # [32,114,56,128]
    XT = [D[0] - D[2], D[1] + D[2], D[2] - D[1], D[1] - D[3]]
    # XT[xi]: [32,114,56,128] -> want [128, n, 114, xi, 56]
    xt6 = np.stack(XT, axis=2)          # [32, 114, 4, 56, 128] fp32
    xt8 = np.stack((XT[0], XT[3]), axis=2)  # [32, 114, 2, 56, 128]
    xt6 = np.ascontiguousarray(
        xt6.transpose(4, 0, 1, 2, 3).astype(np.float16))  # [128,32,114,4,56]
    xt8 = np.ascontiguousarray(
        xt8.transpose(4, 0, 1, 2, 3).astype(F8))          # [128,32,114,2,56]

    in_maps = []
    for c in range(N_CORES):
        sl = slice(c * NPC, (c + 1) * NPC)
        in_maps.append({
            "x8": xt8[:, sl].reshape(CI, NPC, HP, W8),
            "x16": xt6[:, sl].reshape(CI, NPC, HP, W6),
            "w8": w8, "w16": w16,
        })

    nc = _get_nc()
    trace = os.environ.get("BCONV_TRACE", "0") == "1"
    kwargs = {}
    if trace and os.environ.get("BCONV_TRACE_CORES", "") == "all":
        kwargs["trace_cores"] = list(range(N_CORES))
    res = run_bass_kernel_spmd(
        nc, in_maps, core_ids=list(range(N_CORES)), trace=trace, **kwargs
    )
    LAST_RESULT = res

    out = np.empty((32, H, H, CO), np.float32)
    for c in range(N_CORES):
        o = res.results[c]["out_cm"].reshape(CO, NPC, NB * TSB, 2, TROWS, CT)
        # row = 7*T + r ; col = 2*c + phase
        y = o.transpose(1, 2, 4, 5, 3, 0).reshape(NPC, H, H, CO)
        out[c * NPC: (c + 1) * NPC] = y.astype(np.float32)
    return out
